# revision 32
# baseline (speedup 1.0000x reference)
"""AdditiveAttention Trainium2 kernel (8-core data-parallel over batch).

reference:
    q_proj = (query @ Wq + bq)[:, None, :]          # [B, 1, H]
    k_proj = einsum('bsk,kh->bsh', keys, Wk) + bk   # [B, S, H]
    energy = tanh(q_proj + k_proj)                  # [B, S, H]
    scores = einsum('bsh,h->bs', energy, We) + be   # [B, S]
    weights = softmax(scores, axis=-1)              # [B, S]
    context = einsum('bs,bsv->bv', weights, values) # [B, V]
    return (context, weights)

Notes on the device program (per core, BPC=8 batches):
  - keys are pre-transposed on the host to [B, K, S] (and cast to bf16) so
    the contraction dim (k) lands on SBUF partitions with fully-contiguous
    DMA loads; all matmul operands are bf16, accumulation is fp32 in PSUM.
  - energy is produced in [h_part, s_free] layout: lhsT = Wk[k,h] chunks
    (native layout), rhs = keysT[k,s] tiles.  The q_proj+bq+bk bias varies
    along h = partitions, so it is applied as the per-partition bias of the
    ScalarE tanh that evicts PSUM.
  - scores = We . tanh(energy): matmul with lhsT = We column chunk [128,1].
  - softmax skips the max-subtraction: |scores| <= sum|We| ~ 16, exp() is
    safe in fp32, and the shift (incl. the scalar bias `be`) cancels.
  - context = weights @ values: weights row is PE-transposed into [128,1]
    columns used as lhsT against values tiles [128s, 512v].

Scheduling: a software pipeline emits batch b's 2048-matmul energy stream
as the backbone, with two filler queues slotted between 8-matmul energy
groups: (A) the previous s-block's scores matmuls + exp eviction as one
burst (a lone M=1 matmul after an accumulation-group stop pays ~2x300ns of
LDWEIGHTS serialization; back-to-back M=1 matmuls pipeline at full rate),
and (B) batch b-1's softmax/transpose/context items with values-DMA
prepare callbacks running ahead.  Startup DMAs are ordered along the
critical path (wq chain on the scalar HWDGE ring; wk chunks interleaved
with batch 0's first keysT s-tile on the sync ring), and the final batch's
values are prefetched during its own energy phase since its context drain
has nothing left to hide behind.
"""

import numpy as np
import ml_dtypes

import concourse.bacc as bacc
import concourse.tile as tile
import concourse.mybir as mybir
from concourse.bass_utils import run_bass_kernel_spmd

F32 = mybir.dt.float32
BF16 = mybir.dt.bfloat16
AF = mybir.ActivationFunctionType

B, S, D = 64, 2048, 1024
NCORES = 8
BPC = B // NCORES  # batches per core
KC = D // 128  # contraction chunks
HC = D // 128  # h chunks
ST = S // 512  # s tiles of 512
SC = S // 128  # s chunks of 128


def build_program():
    nc = bacc.Bacc("TRN2", target_bir_lowering=False, debug=False)

    keysT_d = nc.dram_tensor("keysT", [BPC, D, S], BF16, kind="ExternalInput")
    values_d = nc.dram_tensor("values", [BPC, S, D], BF16, kind="ExternalInput")
    wk_d = nc.dram_tensor("wk", [D, D], BF16, kind="ExternalInput")
    wq_d = nc.dram_tensor("wq", [D, D], BF16, kind="ExternalInput")
    bb_d = nc.dram_tensor("bb_pack", [128, 2 * HC], F32, kind="ExternalInput")
    wq2_d = nc.dram_tensor("weq_pack", [128, HC + KC * BPC], BF16, kind="ExternalInput")
    ctx_d = nc.dram_tensor("context", [BPC, D], F32, kind="ExternalOutput")
    wout_d = nc.dram_tensor("weights", [BPC, S], F32, kind="ExternalOutput")

    with tile.TileContext(nc) as tc:
        with (
            tc.tile_pool(name="singles", bufs=1) as singles,
            tc.tile_pool(name="kt", bufs=2) as kt_pool,
            tc.tile_pool(name="vt", bufs=18) as vt_pool,
            tc.tile_pool(name="te", bufs=12) as te_pool,
            tc.tile_pool(name="rows", bufs=3) as rows,
            tc.tile_pool(name="small", bufs=3) as small,
            tc.tile_pool(name="psum_e", bufs=3, space="PSUM") as psum_e,
            tc.tile_pool(name="psum_s", bufs=1, space="PSUM") as psum_s,
            tc.tile_pool(name="psum_wt", bufs=1, space="PSUM") as psum_wt,
            tc.tile_pool(name="psum_c", bufs=1, space="PSUM") as psum_c,
        ):
            # ---- one-time setup -------------------------------------------
            # wk + the first keysT tile are emitted first so the energy
            # matmul stream starts as early as possible.
            # The startup is DMA-latency-bound: interleave the wk chunks
            # with the first s-tile of batch 0's keys (in kc order, matching
            # the first energy group's consumption order), and only then
            # queue the rest of batch 0's keys.  The q-projection chain
            # (wq -> qproj matmuls -> qtot) rides the other HWDGE ring.
            wq_sb = singles.tile([128, KC, D], BF16, tag="wq")
            nc.scalar.dma_start(
                out=wq_sb, in_=wq_d.rearrange("(qc p) h -> p qc h", p=128)
            )
            wk_sb = singles.tile([128, KC, D], BF16, tag="wk")
            wk_r = wk_d.rearrange("(kc p) h -> p kc h", p=128)
            kts = {}
            kt0 = kt_pool.tile([128, KC, S], BF16, tag="kt", name="kt")
            kts[0] = kt0
            for kc in range(KC):
                nc.sync.dma_start(out=wk_sb[:, kc, :], in_=wk_r[:, kc, :])
                nc.sync.dma_start(
                    out=kt0[:, kc, 0:512],
                    in_=keysT_d[0, kc * 128 : (kc + 1) * 128, 0:512],
                )

            def load_kt(b, kcs=range(KC)):
                kt = kts.get(b)
                if kt is None:
                    kt = kt_pool.tile([128, KC, S], BF16, tag="kt", name="kt")
                    kts[b] = kt
                for kc in kcs:
                    nc.scalar.dma_start(
                        out=kt[:, kc, :],
                        in_=keysT_d[b, kc * 128 : (kc + 1) * 128, :],
                    )

            weq_sb = singles.tile([128, HC + KC * BPC], BF16, tag="weq")
            nc.scalar.dma_start(out=weq_sb, in_=wq2_d[:])
            we_sb = weq_sb[:, 0:HC]
            qT_sb = weq_sb[:, HC:].rearrange("p (qc b) -> p qc b", b=BPC)
            bb_sb = singles.tile([128, 2 * HC], F32, tag="bb")
            nc.scalar.dma_start(out=bb_sb, in_=bb_d[:])

            bqbk = singles.tile([128, HC], F32, tag="bqbk")
            nc.vector.tensor_add(
                out=bqbk, in0=bb_sb[:, 0:HC], in1=bb_sb[:, HC : 2 * HC]
            )

            ident11 = singles.tile([1, 1], F32, tag="ident")
            nc.vector.memset(ident11, 1.0)

            # q_total[h, b] = (Wq.T @ query.T)[h, b] + bq[h] + bk[h]
            qtot = singles.tile([128, HC, BPC], F32, tag="qtot")
            for hc in range(HC):
                pq = psum_e.tile([128, 512], F32, tag="pe")
                for qc in range(KC):
                    nc.tensor.matmul(
                        pq[:, 0:BPC],
                        lhsT=wq_sb[:, qc, hc * 128 : (hc + 1) * 128],
                        rhs=qT_sb[:, qc, :],
                        start=(qc == 0),
                        stop=(qc == KC - 1),
                    )
                nc.scalar.activation(
                    out=qtot[:, hc, :],
                    in_=pq[:, 0:BPC],
                    func=AF.Identity,
                    bias=bqbk[:, hc : hc + 1],
                    scale=1.0,
                )

            # rest of batch 0's keys (s = 512..2048) as one contiguous-row
            # transfer per kc chunk
            for kc in range(KC):
                nc.sync.dma_start(
                    out=kt0[:, kc, 512:S],
                    in_=keysT_d[0, kc * 128 : (kc + 1) * 128, 512:S],
                )

            # ---- per-batch state ------------------------------------------
            state = {}

            # Two filler queues drained between the 8-matmul energy groups so
            # small PE items never stall the PE on a fresh dependency:
            #   queue A (scoreq): this batch's scores matmuls + psum
            #     evictions, kept exactly ~2 groups behind their tanh.
            #   queue B (tailq): previous batch's softmax/transpose/context
            #     items, paced at one per group with DMA `prepare` callbacks
            #     issued several items ahead.
            scoreq = []
            tailq = []

            PREP_AHEAD = 2

            def flush_tail_one():
                if not tailq:
                    return
                for item in tailq[:PREP_AHEAD]:
                    if item[0] is not None:
                        item[0]()
                        item[0] = None
                item = tailq.pop(0)
                if item[0] is not None:
                    item[0]()
                item[1]()

            def flush_scores():
                # back-to-back M=1 matmuls pipeline at full rate; a lone M=1
                # matmul right after an accumulation-group stop pays ~2x300ns
                # of LDWEIGHTS serialization, so the previous block's scores
                # are emitted as one burst.
                while scoreq:
                    scoreq.pop(0)()

            def flush_all():
                flush_scores()
                while tailq:
                    flush_tail_one()

            def emit_energy_scores(b, st, kt):
                """Energy matmul groups for s-tile st of batch b; the scores
                matmuls and the psum eviction are queued and emitted as a
                burst inside the NEXT block (they depend on this block's ACT
                tanh evictions)."""
                prev_burst = scoreq[:]
                scoreq.clear()
                ps = psum_s.tile([1, 512], F32, tag="ps")
                for hc in range(HC):
                    pe = psum_e.tile([128, 512], F32, tag="pe")
                    for kc in range(KC):
                        nc.tensor.matmul(
                            pe,
                            lhsT=wk_sb[:, kc, hc * 128 : (hc + 1) * 128],
                            rhs=kt[:, kc, st * 512 : (st + 1) * 512],
                            start=(kc == 0),
                            stop=(kc == KC - 1),
                        )
                    te = te_pool.tile([128, 512], BF16, tag="te")
                    nc.scalar.activation(
                        out=te,
                        in_=pe,
                        func=AF.Tanh,
                        bias=qtot[:, hc, b : b + 1],
                        scale=1.0,
                    )

                    def score_mm(te=te, ps=ps, hc=hc):
                        nc.tensor.matmul(
                            ps,
                            lhsT=we_sb[:, hc : hc + 1],
                            rhs=te,
                            start=(hc == 0),
                            stop=(hc == HC - 1),
                        )

                    scoreq.append(score_mm)
                    if hc == 1:
                        for fn in prev_burst:  # previous block's scores + exp
                            fn()
                        prev_burst = []
                    elif hc >= 3 and hc % 2 == 1:
                        flush_tail_one()

                st_b = state[b]

                def exp_evict(b=b, st=st, ps=ps, st_b=st_b):
                    nc.scalar.activation(
                        out=st_b["sexp"][0:1, st * 512 : (st + 1) * 512],
                        in_=ps,
                        func=AF.Exp,
                        accum_out=st_b["dparts"][0:1, st : st + 1],
                    )

                scoreq.append(exp_evict)

            def emit_softmax(b):
                """Normalize scores of batch b; write weights out."""
                st_b = state[b]
                den = small.tile([1, 1], F32, tag="den")
                nc.vector.tensor_reduce(
                    out=den,
                    in_=st_b["dparts"],
                    axis=mybir.AxisListType.X,
                    op=mybir.AluOpType.add,
                )
                rden = small.tile([1, 1], F32, tag="rden")
                nc.vector.reciprocal(out=rden, in_=den)
                wrow = rows.tile([1, S], F32, tag="wrow", bufs=2)
                nc.vector.tensor_scalar_mul(out=wrow, in0=st_b["sexp"], scalar1=rden)
                nc.sync.dma_start(out=wout_d[b : b + 1, :], in_=wrow)
                st_b["wrow"] = wrow
                st_b["pwt"] = psum_wt.tile([128, SC], F32, tag="pwt", name="pwt")
                st_b["wt"] = small.tile([128, SC], BF16, tag="wt", name="wt")
                st_b["pc"] = psum_c.tile([1, D], F32, tag="pc", name="pc")

            def emit_wt_group(b, g):
                """Transpose 4 weight columns of batch b (group g of 4)."""
                st_b = state[b]
                for sc in range(4 * g, 4 * g + 4):
                    nc.tensor.transpose(
                        st_b["pwt"][:, sc : sc + 1],
                        st_b["wrow"][0:1, sc * 128 : (sc + 1) * 128],
                        ident11,
                    )
                nc.scalar.copy(
                    out=st_b["wt"][:, 4 * g : 4 * g + 4],
                    in_=st_b["pwt"][:, 4 * g : 4 * g + 4],
                )

            def emit_values_dma(b, sc):
                vt = vt_pool.tile([128, D], BF16, tag="vt")
                nc.sync.dma_start(
                    out=vt, in_=values_d[b, sc * 128 : (sc + 1) * 128, :]
                )
                state[b]["vts"][sc] = vt

            def emit_context_chunk(b, sc):
                """Context matmuls for s-chunk sc of batch b."""
                st_b = state[b]
                vt = st_b["vts"][sc]
                for vh in range(2):
                    nc.tensor.matmul(
                        st_b["pc"][0:1, vh * 512 : (vh + 1) * 512],
                        lhsT=st_b["wt"][:, sc : sc + 1],
                        rhs=vt[:, vh * 512 : (vh + 1) * 512],
                        start=(sc == 0),
                        stop=(sc == SC - 1),
                    )

            def emit_context_out(b):
                st_b = state[b]
                crow = rows.tile([1, D], F32, tag="crow", bufs=2)
                nc.scalar.copy(out=crow, in_=st_b["pc"])
                nc.sync.dma_start(out=ctx_d[b : b + 1, :], in_=crow)
                del state[b]

            def enqueue_batch_tail(b):
                """Queue softmax-dependent PE work of batch b as fillers."""
                tailq.append([None, lambda b=b: emit_softmax(b)])

                def ctx_group(b, g):
                    for sc in range(4 * g, 4 * g + 4):
                        emit_context_chunk(b, sc)

                def values_group(b, g):
                    for sc in range(4 * g, 4 * g + 4):
                        emit_values_dma(b, sc)

                for g in range(ST):
                    tailq.append([None, lambda b=b, g=g: emit_wt_group(b, g)])
                for g in range(0, ST, 2):
                    tailq.append(
                        [
                            lambda b=b, g=g: (values_group(b, g), values_group(b, g + 1)),
                            lambda b=b, g=g: (ctx_group(b, g), ctx_group(b, g + 1)),
                        ]
                    )
                tailq.append([None, lambda b=b: emit_context_out(b)])

            # Software-pipelined emission: batch b's energy groups carry the
            # filler queue, which holds batch b-1's softmax/context work (and
            # b's own scores matmuls), so the PE never waits on the DVE
            # softmax chain or the values DMA stream, and the M=1 matmuls /
            # transposes interleave with the energy stream instead of
            # clumping (16 back-to-back transposes would let the PE HAM
            # clock gate re-throttle).
            last = BPC - 1
            for b in range(BPC):
                state[b] = {
                    "sexp": rows.tile([1, S], F32, tag="sexp", name="sexp"),
                    "dparts": small.tile([1, ST], F32, tag="dparts", name="dparts"),
                    "vts": [None] * SC,
                }
                for st in range(ST):
                    emit_energy_scores(b, st, kts[b])
                    if st == 0 and b + 1 < BPC:
                        load_kt(b + 1, range(0, KC // 2))
                    if st == 1 and b + 1 < BPC:
                        load_kt(b + 1, range(KC // 2, KC))
                    if b == BPC - 1:
                        # prefetch the final batch's values during its energy
                        # phase (its context work drains at the end with
                        # nothing left to hide the transfers behind)
                        for sc in range(st * 4, st * 4 + 4):
                            emit_values_dma(b, sc)
                if b > 0:
                    kts.pop(b - 1, None)
                    enqueue_batch_tail(b - 1)
            enqueue_batch_tail(last)
            flush_all()

    nc.compile()
    return nc


_PROGRAM = None


def _get_program():
    global _PROGRAM
    if _PROGRAM is None:
        _PROGRAM = build_program()
    return _PROGRAM


def _marshal(inputs):
    return _build_in_maps(
        **{
            k: inputs[k]
            for k in ("query", "keys", "values", "Wq", "bq", "Wk", "bk", "We")
        }
    )


def _build_in_maps(query, keys, values, Wq, bq, Wk, bk, We):
    query = np.asarray(query, dtype=np.float32)
    keys = np.asarray(keys, dtype=np.float32)
    values = np.asarray(values, dtype=np.float32)
    Wq = np.asarray(Wq, dtype=np.float32)
    bq = np.asarray(bq, dtype=np.float32)
    Wk = np.asarray(Wk, dtype=np.float32)
    bk = np.asarray(bk, dtype=np.float32)
    We = np.asarray(We, dtype=np.float32)

    bf16 = ml_dtypes.bfloat16
    keysT = np.ascontiguousarray(keys.transpose(0, 2, 1)).astype(bf16)  # [B,K,S]
    values_b = values.astype(bf16)
    wk_b = Wk.astype(bf16)
    wq_b = Wq.astype(bf16)
    bq_col = np.ascontiguousarray(bq.reshape(HC, 128).T)  # [128, HC]
    bk_col = np.ascontiguousarray(bk.reshape(HC, 128).T)
    bb_pack = np.concatenate([bq_col, bk_col], axis=1)  # [128, 2*HC] f32
    we_col = We.reshape(HC, 128).T.astype(bf16)  # [128, HC]

    in_maps = []
    for c in range(NCORES):
        bs = slice(c * BPC, (c + 1) * BPC)
        # q_pack[p, qc*BPC + b] = query[bs][b, qc*128 + p]
        q_pack = (
            query[bs].T.reshape(KC, 128, BPC).transpose(1, 0, 2).reshape(128, -1)
        ).astype(bf16)
        weq_pack = np.ascontiguousarray(np.concatenate([we_col, q_pack], axis=1))
        in_maps.append(
            {
                "keysT": keysT[bs],
                "values": values_b[bs],
                "wk": wk_b,
                "wq": wq_b,
                "bb_pack": bb_pack,
                "weq_pack": weq_pack,
            }
        )
    return in_maps


def kernel(query, keys, values, Wq, bq, Wk, bk, We, be):
    in_maps = _build_in_maps(query, keys, values, Wq, bq, Wk, bk, We)
    res = _run(in_maps)
    context = np.concatenate([res.results[c]["context"] for c in range(NCORES)], 0)
    weights = np.concatenate([res.results[c]["weights"] for c in range(NCORES)], 0)
    return (context, weights)


def _run(in_maps, **kwargs):
    nc = _get_program()
    return run_bass_kernel_spmd(nc, in_maps, core_ids=list(range(NCORES)), **kwargs)


# revision 33
# speedup vs baseline: 1.0060x; 1.0060x over previous
"""AdditiveAttention Trainium2 kernel (8-core data-parallel over batch).

reference:
    q_proj = (query @ Wq + bq)[:, None, :]          # [B, 1, H]
    k_proj = einsum('bsk,kh->bsh', keys, Wk) + bk   # [B, S, H]
    energy = tanh(q_proj + k_proj)                  # [B, S, H]
    scores = einsum('bsh,h->bs', energy, We) + be   # [B, S]
    weights = softmax(scores, axis=-1)              # [B, S]
    context = einsum('bs,bsv->bv', weights, values) # [B, V]
    return (context, weights)

Notes on the device program (per core, BPC=8 batches):
  - keys are pre-transposed on the host to [B, K, S] (and cast to bf16) so
    the contraction dim (k) lands on SBUF partitions with fully-contiguous
    DMA loads; all matmul operands are bf16, accumulation is fp32 in PSUM.
  - energy is produced in [h_part, s_free] layout: lhsT = Wk[k,h] chunks
    (native layout), rhs = keysT[k,s] tiles.  The q_proj+bq+bk bias varies
    along h = partitions, so it is applied as the per-partition bias of the
    ScalarE tanh that evicts PSUM.
  - scores = We . tanh(energy): matmul with lhsT = We column chunk [128,1].
  - softmax skips the max-subtraction: |scores| <= sum|We| ~ 16, exp() is
    safe in fp32, and the shift (incl. the scalar bias `be`) cancels.
  - context = weights @ values: weights row is PE-transposed into [128,1]
    columns used as lhsT against values tiles [128s, 512v].

Scheduling: a software pipeline emits batch b's 2048-matmul energy stream
as the backbone, with two filler queues slotted between 8-matmul energy
groups: (A) the previous s-block's scores matmuls + exp eviction as one
burst (a lone M=1 matmul after an accumulation-group stop pays ~2x300ns of
LDWEIGHTS serialization; back-to-back M=1 matmuls pipeline at full rate),
and (B) batch b-1's softmax/transpose/context items with values-DMA
prepare callbacks running ahead.  Startup DMAs are ordered along the
critical path (wq chain on the scalar HWDGE ring; wk chunks interleaved
with batch 0's first keysT s-tile on the sync ring), and the final batch's
values are prefetched during its own energy phase since its context drain
has nothing left to hide behind.
"""

import numpy as np
import ml_dtypes

import concourse.bacc as bacc
import concourse.tile as tile
import concourse.mybir as mybir
from concourse.bass_utils import run_bass_kernel_spmd

F32 = mybir.dt.float32
BF16 = mybir.dt.bfloat16
AF = mybir.ActivationFunctionType

B, S, D = 64, 2048, 1024
NCORES = 8
BPC = B // NCORES  # batches per core
KC = D // 128  # contraction chunks
HC = D // 128  # h chunks
ST = S // 512  # s tiles of 512
SC = S // 128  # s chunks of 128


def build_program():
    nc = bacc.Bacc("TRN2", target_bir_lowering=False, debug=False)

    keysT_d = nc.dram_tensor("keysT", [BPC, D, S], BF16, kind="ExternalInput")
    values_d = nc.dram_tensor("values", [BPC, S, D], BF16, kind="ExternalInput")
    wk_d = nc.dram_tensor("wk", [D, D], BF16, kind="ExternalInput")
    wq_d = nc.dram_tensor("wq", [D, D], BF16, kind="ExternalInput")
    bb_d = nc.dram_tensor("bb_pack", [128, 2 * HC], F32, kind="ExternalInput")
    wq2_d = nc.dram_tensor("weq_pack", [128, HC + KC * BPC], BF16, kind="ExternalInput")
    ctx_d = nc.dram_tensor("context", [BPC, D], F32, kind="ExternalOutput")
    wout_d = nc.dram_tensor("weights", [BPC, S], F32, kind="ExternalOutput")

    with tile.TileContext(nc) as tc:
        with (
            tc.tile_pool(name="singles", bufs=1) as singles,
            tc.tile_pool(name="kt", bufs=2) as kt_pool,
            tc.tile_pool(name="vt", bufs=18) as vt_pool,
            tc.tile_pool(name="te", bufs=12) as te_pool,
            tc.tile_pool(name="rows", bufs=3) as rows,
            tc.tile_pool(name="small", bufs=3) as small,
            tc.tile_pool(name="psum_e", bufs=3, space="PSUM") as psum_e,
            tc.tile_pool(name="psum_s", bufs=1, space="PSUM") as psum_s,
            tc.tile_pool(name="psum_wt", bufs=1, space="PSUM") as psum_wt,
            tc.tile_pool(name="psum_c", bufs=1, space="PSUM") as psum_c,
        ):
            # ---- one-time setup -------------------------------------------
            # wk + the first keysT tile are emitted first so the energy
            # matmul stream starts as early as possible.
            # The startup is DMA-latency-bound: interleave the wk chunks
            # with the first s-tile of batch 0's keys (in kc order, matching
            # the first energy group's consumption order), and only then
            # queue the rest of batch 0's keys.  The q-projection chain
            # (wq -> qproj matmuls -> qtot) rides the other HWDGE ring.
            wq_sb = singles.tile([128, KC, D], BF16, tag="wq")
            nc.scalar.dma_start(
                out=wq_sb, in_=wq_d.rearrange("(qc p) h -> p qc h", p=128)
            )
            wk_sb = singles.tile([128, KC, D], BF16, tag="wk")
            wk_r = wk_d.rearrange("(kc p) h -> p kc h", p=128)
            kts = {}
            kt0 = kt_pool.tile([128, KC, S], BF16, tag="kt", name="kt")
            kts[0] = kt0
            for kc in range(KC):
                nc.sync.dma_start(out=wk_sb[:, kc, :], in_=wk_r[:, kc, :])
                nc.sync.dma_start(
                    out=kt0[:, kc, 0:512],
                    in_=keysT_d[0, kc * 128 : (kc + 1) * 128, 0:512],
                )

            def load_kt(b, kcs=range(KC)):
                kt = kts.get(b)
                if kt is None:
                    kt = kt_pool.tile([128, KC, S], BF16, tag="kt", name="kt")
                    kts[b] = kt
                for kc in kcs:
                    nc.sync.dma_start(
                        out=kt[:, kc, :],
                        in_=keysT_d[b, kc * 128 : (kc + 1) * 128, :],
                    )

            weq_sb = singles.tile([128, HC + KC * BPC], BF16, tag="weq")
            nc.scalar.dma_start(out=weq_sb, in_=wq2_d[:])
            we_sb = weq_sb[:, 0:HC]
            qT_sb = weq_sb[:, HC:].rearrange("p (qc b) -> p qc b", b=BPC)
            bb_sb = singles.tile([128, 2 * HC], F32, tag="bb")
            nc.scalar.dma_start(out=bb_sb, in_=bb_d[:])

            bqbk = singles.tile([128, HC], F32, tag="bqbk")
            nc.vector.tensor_add(
                out=bqbk, in0=bb_sb[:, 0:HC], in1=bb_sb[:, HC : 2 * HC]
            )

            ident11 = singles.tile([1, 1], F32, tag="ident")
            nc.vector.memset(ident11, 1.0)

            # q_total[h, b] = (Wq.T @ query.T)[h, b] + bq[h] + bk[h]
            qtot = singles.tile([128, HC, BPC], F32, tag="qtot")
            for hc in range(HC):
                pq = psum_e.tile([128, 512], F32, tag="pe")
                for qc in range(KC):
                    nc.tensor.matmul(
                        pq[:, 0:BPC],
                        lhsT=wq_sb[:, qc, hc * 128 : (hc + 1) * 128],
                        rhs=qT_sb[:, qc, :],
                        start=(qc == 0),
                        stop=(qc == KC - 1),
                    )
                nc.scalar.activation(
                    out=qtot[:, hc, :],
                    in_=pq[:, 0:BPC],
                    func=AF.Identity,
                    bias=bqbk[:, hc : hc + 1],
                    scale=1.0,
                )

            # rest of batch 0's keys (s = 512..2048) as one contiguous-row
            # transfer per kc chunk
            for kc in range(KC):
                nc.sync.dma_start(
                    out=kt0[:, kc, 512:S],
                    in_=keysT_d[0, kc * 128 : (kc + 1) * 128, 512:S],
                )

            # ---- per-batch state ------------------------------------------
            state = {}

            # Two filler queues drained between the 8-matmul energy groups so
            # small PE items never stall the PE on a fresh dependency:
            #   queue A (scoreq): this batch's scores matmuls + psum
            #     evictions, kept exactly ~2 groups behind their tanh.
            #   queue B (tailq): previous batch's softmax/transpose/context
            #     items, paced at one per group with DMA `prepare` callbacks
            #     issued several items ahead.
            scoreq = []
            tailq = []

            PREP_AHEAD = 2

            def flush_tail_one():
                if not tailq:
                    return
                for item in tailq[:PREP_AHEAD]:
                    if item[0] is not None:
                        item[0]()
                        item[0] = None
                item = tailq.pop(0)
                if item[0] is not None:
                    item[0]()
                item[1]()

            def flush_scores():
                # back-to-back M=1 matmuls pipeline at full rate; a lone M=1
                # matmul right after an accumulation-group stop pays ~2x300ns
                # of LDWEIGHTS serialization, so the previous block's scores
                # are emitted as one burst.
                while scoreq:
                    scoreq.pop(0)()

            def flush_all():
                flush_scores()
                while tailq:
                    flush_tail_one()

            def emit_energy_scores(b, st, kt):
                """Energy matmul groups for s-tile st of batch b; the scores
                matmuls and the psum eviction are queued and emitted as a
                burst inside the NEXT block (they depend on this block's ACT
                tanh evictions)."""
                prev_burst = scoreq[:]
                scoreq.clear()
                ps = psum_s.tile([1, 512], F32, tag="ps")
                for hc in range(HC):
                    pe = psum_e.tile([128, 512], F32, tag="pe")
                    for kc in range(KC):
                        nc.tensor.matmul(
                            pe,
                            lhsT=wk_sb[:, kc, hc * 128 : (hc + 1) * 128],
                            rhs=kt[:, kc, st * 512 : (st + 1) * 512],
                            start=(kc == 0),
                            stop=(kc == KC - 1),
                        )
                    te = te_pool.tile([128, 512], BF16, tag="te")
                    nc.scalar.activation(
                        out=te,
                        in_=pe,
                        func=AF.Tanh,
                        bias=qtot[:, hc, b : b + 1],
                        scale=1.0,
                    )

                    def score_mm(te=te, ps=ps, hc=hc):
                        nc.tensor.matmul(
                            ps,
                            lhsT=we_sb[:, hc : hc + 1],
                            rhs=te,
                            start=(hc == 0),
                            stop=(hc == HC - 1),
                        )

                    scoreq.append(score_mm)
                    if hc == 1:
                        for fn in prev_burst:  # previous block's scores + exp
                            fn()
                        prev_burst = []
                    elif hc >= 3 and hc % 2 == 1:
                        flush_tail_one()

                st_b = state[b]

                def exp_evict(b=b, st=st, ps=ps, st_b=st_b):
                    nc.scalar.activation(
                        out=st_b["sexp"][0:1, st * 512 : (st + 1) * 512],
                        in_=ps,
                        func=AF.Exp,
                        accum_out=st_b["dparts"][0:1, st : st + 1],
                    )

                scoreq.append(exp_evict)

            def emit_softmax(b):
                """Normalize scores of batch b; write weights out."""
                st_b = state[b]
                den = small.tile([1, 1], F32, tag="den")
                nc.vector.tensor_reduce(
                    out=den,
                    in_=st_b["dparts"],
                    axis=mybir.AxisListType.X,
                    op=mybir.AluOpType.add,
                )
                rden = small.tile([1, 1], F32, tag="rden")
                nc.vector.reciprocal(out=rden, in_=den)
                wrow = rows.tile([1, S], F32, tag="wrow", bufs=2)
                nc.vector.tensor_scalar_mul(out=wrow, in0=st_b["sexp"], scalar1=rden)
                nc.sync.dma_start(out=wout_d[b : b + 1, :], in_=wrow)
                st_b["wrow"] = wrow
                st_b["pwt"] = psum_wt.tile([128, SC], F32, tag="pwt", name="pwt")
                st_b["wt"] = small.tile([128, SC], BF16, tag="wt", name="wt")
                st_b["pc"] = psum_c.tile([1, D], F32, tag="pc", name="pc")

            def emit_wt_group(b, g):
                """Transpose 4 weight columns of batch b (group g of 4)."""
                st_b = state[b]
                for sc in range(4 * g, 4 * g + 4):
                    nc.tensor.transpose(
                        st_b["pwt"][:, sc : sc + 1],
                        st_b["wrow"][0:1, sc * 128 : (sc + 1) * 128],
                        ident11,
                    )
                nc.scalar.copy(
                    out=st_b["wt"][:, 4 * g : 4 * g + 4],
                    in_=st_b["pwt"][:, 4 * g : 4 * g + 4],
                )

            def emit_values_dma(b, sc):
                vt = vt_pool.tile([128, D], BF16, tag="vt")
                nc.sync.dma_start(
                    out=vt, in_=values_d[b, sc * 128 : (sc + 1) * 128, :]
                )
                state[b]["vts"][sc] = vt

            def emit_context_chunk(b, sc):
                """Context matmuls for s-chunk sc of batch b."""
                st_b = state[b]
                vt = st_b["vts"][sc]
                for vh in range(2):
                    nc.tensor.matmul(
                        st_b["pc"][0:1, vh * 512 : (vh + 1) * 512],
                        lhsT=st_b["wt"][:, sc : sc + 1],
                        rhs=vt[:, vh * 512 : (vh + 1) * 512],
                        start=(sc == 0),
                        stop=(sc == SC - 1),
                    )

            def emit_context_out(b):
                st_b = state[b]
                crow = rows.tile([1, D], F32, tag="crow", bufs=2)
                nc.scalar.copy(out=crow, in_=st_b["pc"])
                nc.sync.dma_start(out=ctx_d[b : b + 1, :], in_=crow)
                del state[b]

            def enqueue_batch_tail(b):
                """Queue softmax-dependent PE work of batch b as fillers."""
                tailq.append([None, lambda b=b: emit_softmax(b)])

                def ctx_group(b, g):
                    for sc in range(4 * g, 4 * g + 4):
                        emit_context_chunk(b, sc)

                def values_group(b, g):
                    for sc in range(4 * g, 4 * g + 4):
                        emit_values_dma(b, sc)

                for g in range(ST):
                    tailq.append([None, lambda b=b, g=g: emit_wt_group(b, g)])
                for g in range(0, ST, 2):
                    tailq.append(
                        [
                            lambda b=b, g=g: (values_group(b, g), values_group(b, g + 1)),
                            lambda b=b, g=g: (ctx_group(b, g), ctx_group(b, g + 1)),
                        ]
                    )
                tailq.append([None, lambda b=b: emit_context_out(b)])

            # Software-pipelined emission: batch b's energy groups carry the
            # filler queue, which holds batch b-1's softmax/context work (and
            # b's own scores matmuls), so the PE never waits on the DVE
            # softmax chain or the values DMA stream, and the M=1 matmuls /
            # transposes interleave with the energy stream instead of
            # clumping (16 back-to-back transposes would let the PE HAM
            # clock gate re-throttle).
            last = BPC - 1
            for b in range(BPC):
                state[b] = {
                    "sexp": rows.tile([1, S], F32, tag="sexp", name="sexp"),
                    "dparts": small.tile([1, ST], F32, tag="dparts", name="dparts"),
                    "vts": [None] * SC,
                }
                for st in range(ST):
                    emit_energy_scores(b, st, kts[b])
                    if st == 0 and b + 1 < BPC:
                        load_kt(b + 1, range(0, KC // 2))
                    if st == 1 and b + 1 < BPC:
                        load_kt(b + 1, range(KC // 2, KC))
                    if b == BPC - 1:
                        # prefetch the final batch's values during its energy
                        # phase (its context work drains at the end with
                        # nothing left to hide the transfers behind)
                        for sc in range(st * 4, st * 4 + 4):
                            emit_values_dma(b, sc)
                if b > 0:
                    kts.pop(b - 1, None)
                    enqueue_batch_tail(b - 1)
            enqueue_batch_tail(last)
            flush_all()

    nc.compile()
    return nc


_PROGRAM = None


def _get_program():
    global _PROGRAM
    if _PROGRAM is None:
        _PROGRAM = build_program()
    return _PROGRAM


def _marshal(inputs):
    return _build_in_maps(
        **{
            k: inputs[k]
            for k in ("query", "keys", "values", "Wq", "bq", "Wk", "bk", "We")
        }
    )


def _build_in_maps(query, keys, values, Wq, bq, Wk, bk, We):
    query = np.asarray(query, dtype=np.float32)
    keys = np.asarray(keys, dtype=np.float32)
    values = np.asarray(values, dtype=np.float32)
    Wq = np.asarray(Wq, dtype=np.float32)
    bq = np.asarray(bq, dtype=np.float32)
    Wk = np.asarray(Wk, dtype=np.float32)
    bk = np.asarray(bk, dtype=np.float32)
    We = np.asarray(We, dtype=np.float32)

    bf16 = ml_dtypes.bfloat16
    keysT = np.ascontiguousarray(keys.transpose(0, 2, 1)).astype(bf16)  # [B,K,S]
    values_b = values.astype(bf16)
    wk_b = Wk.astype(bf16)
    wq_b = Wq.astype(bf16)
    bq_col = np.ascontiguousarray(bq.reshape(HC, 128).T)  # [128, HC]
    bk_col = np.ascontiguousarray(bk.reshape(HC, 128).T)
    bb_pack = np.concatenate([bq_col, bk_col], axis=1)  # [128, 2*HC] f32
    we_col = We.reshape(HC, 128).T.astype(bf16)  # [128, HC]

    in_maps = []
    for c in range(NCORES):
        bs = slice(c * BPC, (c + 1) * BPC)
        # q_pack[p, qc*BPC + b] = query[bs][b, qc*128 + p]
        q_pack = (
            query[bs].T.reshape(KC, 128, BPC).transpose(1, 0, 2).reshape(128, -1)
        ).astype(bf16)
        weq_pack = np.ascontiguousarray(np.concatenate([we_col, q_pack], axis=1))
        in_maps.append(
            {
                "keysT": keysT[bs],
                "values": values_b[bs],
                "wk": wk_b,
                "wq": wq_b,
                "bb_pack": bb_pack,
                "weq_pack": weq_pack,
            }
        )
    return in_maps


def kernel(query, keys, values, Wq, bq, Wk, bk, We, be):
    in_maps = _build_in_maps(query, keys, values, Wq, bq, Wk, bk, We)
    res = _run(in_maps)
    context = np.concatenate([res.results[c]["context"] for c in range(NCORES)], 0)
    weights = np.concatenate([res.results[c]["weights"] for c in range(NCORES)], 0)
    return (context, weights)


def _run(in_maps, **kwargs):
    nc = _get_program()
    return run_bass_kernel_spmd(nc, in_maps, core_ids=list(range(NCORES)), **kwargs)
